# revision 6
# baseline (speedup 1.0000x reference)
"""AttentionDecoder step (batch=1) on 8 TRN2 NeuronCores, tensor-parallel.

Layers: emb lookup -> attention over 15 encoder positions -> combine Linear
(relu) -> GRU cell -> out Linear + log_softmax.

Sharding (8 cores): column-parallel on every big Linear. Core r owns rows
[512r, 512r+512) of comb_W, of each GRU gate (r/z/n), and of out_W. The tiny
[1, 512] per-core activations (x, h_new, logits) are all-gathered on device.

Memory regime: ~604MB of f32 weights stream once per call (~76MB/core).
The host pre-transposes weights to [K, M] layout so the TensorEngine can
stream them as the moving operand against a stationary activation-column
chunk ([128, 1]); psum accumulates [1, 512] rows over K chunks.

Activation layout convention: a length-N vector v is held either as a
"row" [1, N] on one partition, or as "cols" [128, N/128] with
cols[p, c] = v[128c + p]. Host-provided vectors (emb, h0) arrive as cols;
device-computed rows are converted to cols via AllGather + PE transpose.
"""

import os
import sys

sys.path.insert(0, "/opt/trn_rl_repo")

import numpy as np

import concourse.bass as bass
import concourse.mybir as mybir
import concourse.tile as tile
from concourse import bacc
from concourse.bass_utils import run_bass_kernel_spmd
from concourse.masks import make_identity

H = 4096
O = 4096
MAX_LEN = 15
N_CORES = 8
SH = H // N_CORES          # 512, per-core shard of any 4096-dim output
NCH = H // 128             # 32 chunks of 128 in a 4096 vector
DT = mybir.dt.float32
DTR = mybir.dt.float32r
AF = mybir.ActivationFunctionType
ALU = mybir.AluOpType
RG = [list(range(N_CORES))]

_NC_CACHE = None


def _emit_mv(nc, psum_ap, wtile, rows_per_tile, t, lhsT_of_chunk, width, k_chunks_total,
             tile_chunk0):
    """Matmuls for one weight tile: psum[1, width] += lhsT_c.T @ w[:, slice]."""
    cpt = rows_per_tile // 128
    for cl in range(cpt):
        c = tile_chunk0 + cl
        nc.tensor.matmul(
            psum_ap,
            lhsT_of_chunk(c),
            wtile[:, cl, 0:width],
            start=(c == 0),
            stop=(c == k_chunks_total - 1),
        )


def _build_nc():
    nc = bacc.Bacc("TRN2", target_bir_lowering=False, debug=False,
                   num_devices=N_CORES)

    # ---- I/O ----
    emb_cols_i = nc.dram_tensor("emb_cols", [128, NCH], DTR, kind="ExternalInput")
    h0_cols_i = nc.dram_tensor("h0_cols", [128, NCH], DTR, kind="ExternalInput")
    h0_shard_i = nc.dram_tensor("h0_shard", [1, SH], DT, kind="ExternalInput")
    e_seq_i = nc.dram_tensor("e_seq", [MAX_LEN, H], DT, kind="ExternalInput")
    att_wt_i = nc.dram_tensor("att_wt", [2 * H, 16], DTR, kind="ExternalInput")
    att_b_i = nc.dram_tensor("att_b", [1, 16], DT, kind="ExternalInput")
    comb_wt_i = nc.dram_tensor("comb_wt", [2 * H, SH], DTR, kind="ExternalInput")
    comb_b_i = nc.dram_tensor("comb_b", [1, SH], DT, kind="ExternalInput")
    rz_wt_i = nc.dram_tensor("rz_wt", [2 * H, 2 * SH], DTR, kind="ExternalInput")
    rz_b_i = nc.dram_tensor("rz_b", [1, 2 * SH], DT, kind="ExternalInput")
    in_wt_i = nc.dram_tensor("in_wt", [H, SH], DTR, kind="ExternalInput")
    in_b_i = nc.dram_tensor("in_b", [1, SH], DT, kind="ExternalInput")
    hn_wt_i = nc.dram_tensor("hn_wt", [H, SH], DTR, kind="ExternalInput")
    hn_b_i = nc.dram_tensor("hn_b", [1, SH], DT, kind="ExternalInput")
    out_wt_i = nc.dram_tensor("out_wt", [H, SH], DTR, kind="ExternalInput")
    out_b_i = nc.dram_tensor("out_b", [1, SH], DT, kind="ExternalInput")

    out_ext = nc.dram_tensor("out_sm", [NCH, 128], DT, kind="ExternalOutput")
    hnew_ext = nc.dram_tensor("h_new_full", [NCH, 128], DT, kind="ExternalOutput")
    attw_ext = nc.dram_tensor("att_w", [1, MAX_LEN], DT, kind="ExternalOutput")

    with tile.TileContext(nc) as tc:
        with (
            tc.tile_pool(name="wpool", bufs=3) as wpool,
            tc.tile_pool(name="sb", bufs=1) as sb,
            tc.tile_pool(name="ps", bufs=1, space="PSUM") as ps,
            tc.tile_pool(name="dram", bufs=1, space="DRAM") as dram,
        ):
            # ---- small resident loads ----
            e_seq = sb.tile([MAX_LEN, H], DT, tag="e_seq")
            nc.sync.dma_start(e_seq[:], e_seq_i[:])
            att_sb = sb.tile([128, 64 * 16], DTR, tag="att_sb")
            for c in range(64):
                nc.sync.dma_start(att_sb[:, 16 * c:16 * (c + 1)],
                                  att_wt_i[128 * c:128 * (c + 1), :])
            emb_cols = sb.tile([128, NCH], DTR, tag="emb_cols")
            nc.sync.dma_start(emb_cols[:], emb_cols_i[:])
            h0_cols = sb.tile([128, NCH], DTR, tag="h0_cols")
            nc.sync.dma_start(h0_cols[:], h0_cols_i[:])
            small_rows = {}
            for name, t_in, width in (
                ("att_b", att_b_i, 16), ("comb_b", comb_b_i, SH),
                ("rz_b", rz_b_i, 2 * SH), ("in_b", in_b_i, SH),
                ("hn_b", hn_b_i, SH), ("out_b", out_b_i, SH),
                ("h0_shard", h0_shard_i, SH),
            ):
                tl = sb.tile([1, width], DT, tag=name)
                nc.sync.dma_start(tl[:], t_in[:])
                small_rows[name] = tl

            ident32 = sb.tile([32, 32], DT, tag="ident32")
            make_identity(nc, ident32[:])
            ident1 = sb.tile([1, 1], DT, tag="ident1")
            nc.gpsimd.memset(ident1[:], 1.0)
            ones32 = sb.tile([1, 32], DT, tag="ones32")
            nc.gpsimd.memset(ones32[:], 1.0)

            # dram bounce buffers for collectives
            ag_x_in = dram.tile([4, 128], DT, tag="ag_x_in")
            ag_x_out = dram.tile([NCH, 128], DT, tag="ag_x_out")
            ag_h_in = dram.tile([4, 128], DT, tag="ag_h_in")
            ag_h_out = dram.tile([NCH, 128], DT, tag="ag_h_out")
            ag_l_in = dram.tile([4, 128], DT, tag="ag_l_in")
            ag_l_out = dram.tile([NCH, 128], DT, tag="ag_l_out")

            def eh_chunk(c):
                return emb_cols[:, c - 0:c + 1] if c < NCH else h0_cols[:, c - NCH:c - NCH + 1]

            # ---- attention logits: [1,16] = sum_c eh_c.T @ att_sb_c ----
            p_attl = ps.tile([1, 16], DT, tag="p_t")
            for c in range(64):
                nc.tensor.matmul(p_attl[:], eh_chunk(c), att_sb[:, 16 * c:16 * (c + 1)],
                                 start=(c == 0), stop=(c == 63))

            # softmax over the first 15 entries (row layout, 1 partition)
            attl_row = sb.tile([1, MAX_LEN], DT, tag="attl_row")
            nc.vector.tensor_add(attl_row[:], p_attl[:, 0:MAX_LEN],
                                 small_rows["att_b"][:, 0:MAX_LEN])
            mx = sb.tile([1, 1], DT, tag="attl_mx")
            nc.vector.reduce_max(mx[:], attl_row[:], axis=mybir.AxisListType.X)
            nmx = sb.tile([1, 1], DT, tag="attl_nmx")
            nc.vector.tensor_scalar_mul(nmx[:], mx[:], -1.0)
            expr = sb.tile([1, MAX_LEN], DT, tag="attl_exp")
            sume = sb.tile([1, 1], DT, tag="attl_sum")
            nc.scalar.activation(expr[:], attl_row[:], AF.Exp, bias=nmx[:],
                                 accum_out=sume[:])
            rsum = sb.tile([1, 1], DT, tag="attl_rsum")
            nc.vector.reciprocal(rsum[:], sume[:])
            attw_row = sb.tile([1, MAX_LEN], DT, tag="attw_row")
            nc.vector.tensor_scalar_mul(attw_row[:], expr[:], rsum[:])
            nc.sync.dma_start(attw_ext[:], attw_row[:])

            # transpose att weights row -> column [15, 1]
            p_awc = ps.tile([MAX_LEN, 1], DT, tag="p_t")
            nc.tensor.transpose(p_awc[:], attw_row[:], ident1[:])
            aw_col = sb.tile([MAX_LEN, 1], DT, tag="aw_col")
            nc.vector.tensor_copy(aw_col[:], p_awc[:])

            # att_applied as columns: [128, 32], chunk j = e_seq[:, 128j:].T @ aw
            p_attc = ps.tile([128, NCH], DT, tag="p_t")
            for j in range(NCH):
                nc.tensor.matmul(p_attc[:, j:j + 1], e_seq[:, 128 * j:128 * (j + 1)],
                                 aw_col[:], start=True, stop=True)
            att_cols = sb.tile([128, NCH], DTR, tag="att_cols")
            nc.vector.tensor_copy(att_cols[:], p_attc[:])

            def comb_chunk(c):
                return emb_cols[:, c:c + 1] if c < NCH else att_cols[:, c - NCH:c - NCH + 1]

            # ---- combine Linear: x = relu(combined @ comb_W.T + b), sharded ----
            p_x = ps.tile([1, SH], DT, tag="p_xl")
            for t in range(4):
                wt = wpool.tile([128, 16, SH], DTR, tag="wt")
                nc.sync.dma_start(
                    wt[:], comb_wt_i[2048 * t:2048 * (t + 1), :]
                    .rearrange("(c p) n -> p c n", p=128))
                _emit_mv(nc, p_x[:], wt, 2048, t, comb_chunk, SH, 64, 16 * t)
            x_row = sb.tile([1, SH], DT, tag="x_row")
            nc.vector.tensor_add(x_row[:], p_x[:], small_rows["comb_b"][:])
            nc.scalar.activation(x_row[:], x_row[:], AF.Relu)

            # ---- AllGather x -> x_cols [128, 32] ----
            for i in range(4):
                nc.sync.dma_start(ag_x_in[i:i + 1, :], x_row[0:1, 128 * i:128 * (i + 1)])
            nc.gpsimd.collective_compute(
                "AllGather", ALU.bypass, replica_groups=RG,
                ins=[ag_x_in[:].opt()], outs=[ag_x_out[:].opt()])
            x_rows_full = sb.tile([NCH, 128], DT, tag="x_rows_full")
            nc.sync.dma_start(x_rows_full[:], ag_x_out[:])
            p_xc = ps.tile([128, NCH], DT, tag="p_t")
            nc.tensor.transpose(p_xc[:], x_rows_full[:], ident32[:])
            x_cols = sb.tile([128, NCH], DTR, tag="x_cols")
            nc.vector.tensor_copy(x_cols[:], p_xc[:])

            def xh_chunk(c):
                return x_cols[:, c:c + 1] if c < NCH else h0_cols[:, c - NCH:c - NCH + 1]

            # ---- GRU hh n-gate first (only needs h0) ----
            p_hn = ps.tile([1, SH], DT, tag="p_hn")
            for t in range(2):
                wt = wpool.tile([128, 16, SH], DTR, tag="wt")
                nc.sync.dma_start(
                    wt[:], hn_wt_i[2048 * t:2048 * (t + 1), :]
                    .rearrange("(c p) n -> p c n", p=128))
                _emit_mv(nc, p_hn[:], wt, 2048, t,
                         lambda c: h0_cols[:, c:c + 1], SH, 32, 16 * t)

            # ---- GRU r,z gates (stacked [x; h0] contraction) ----
            p_r = ps.tile([1, SH], DT, tag="p_r")
            p_z = ps.tile([1, SH], DT, tag="p_z")
            for t in range(8):
                wt = wpool.tile([128, 8, 2 * SH], DTR, tag="wt")
                nc.sync.dma_start(
                    wt[:], rz_wt_i[1024 * t:1024 * (t + 1), :]
                    .rearrange("(c p) n -> p c n", p=128))
                for cl in range(8):
                    c = 8 * t + cl
                    nc.tensor.matmul(p_r[:], xh_chunk(c),
                                     wt[:, cl, 0:SH],
                                     start=(c == 0), stop=(c == 63))
                    nc.tensor.matmul(p_z[:], xh_chunk(c),
                                     wt[:, cl, SH:2 * SH],
                                     start=(c == 0), stop=(c == 63))

            # ---- GRU ih n-gate (needs x) ----
            p_in = ps.tile([1, SH], DT, tag="p_in")
            for t in range(2):
                wt = wpool.tile([128, 16, SH], DTR, tag="wt")
                nc.sync.dma_start(
                    wt[:], in_wt_i[2048 * t:2048 * (t + 1), :]
                    .rearrange("(c p) n -> p c n", p=128))
                _emit_mv(nc, p_in[:], wt, 2048, t,
                         lambda c: x_cols[:, c:c + 1], SH, 32, 16 * t)

            # ---- GRU cell elementwise (row layout, 1 partition) ----
            r_row = sb.tile([1, SH], DT, tag="r_row")
            nc.vector.tensor_add(r_row[:], p_r[:], small_rows["rz_b"][:, 0:SH])
            nc.scalar.activation(r_row[:], r_row[:], AF.Sigmoid)
            z_row = sb.tile([1, SH], DT, tag="z_row")
            nc.vector.tensor_add(z_row[:], p_z[:], small_rows["rz_b"][:, SH:2 * SH])
            nc.scalar.activation(z_row[:], z_row[:], AF.Sigmoid)
            hn_row = sb.tile([1, SH], DT, tag="hn_row")
            nc.vector.tensor_add(hn_row[:], p_hn[:], small_rows["hn_b"][:])
            rhn = sb.tile([1, SH], DT, tag="rhn")
            nc.vector.tensor_mul(rhn[:], r_row[:], hn_row[:])
            n_row = sb.tile([1, SH], DT, tag="n_row")
            nc.vector.tensor_add(n_row[:], p_in[:], small_rows["in_b"][:])
            nc.vector.tensor_add(n_row[:], n_row[:], rhn[:])
            nc.scalar.activation(n_row[:], n_row[:], AF.Tanh)
            # h_new = n + z * (h0 - n)
            d_row = sb.tile([1, SH], DT, tag="d_row")
            nc.vector.tensor_sub(d_row[:], small_rows["h0_shard"][:], n_row[:])
            nc.vector.tensor_mul(d_row[:], z_row[:], d_row[:])
            hnew_row = sb.tile([1, SH], DT, tag="hnew_row")
            nc.vector.tensor_add(hnew_row[:], n_row[:], d_row[:])

            # ---- AllGather h_new ----
            for i in range(4):
                nc.sync.dma_start(ag_h_in[i:i + 1, :], hnew_row[0:1, 128 * i:128 * (i + 1)])
            nc.gpsimd.collective_compute(
                "AllGather", ALU.bypass, replica_groups=RG,
                ins=[ag_h_in[:].opt()], outs=[ag_h_out[:].opt()])
            h_rows_full = sb.tile([NCH, 128], DT, tag="h_rows_full")
            nc.sync.dma_start(h_rows_full[:], ag_h_out[:])
            nc.sync.dma_start(hnew_ext[:], h_rows_full[:])
            p_hc = ps.tile([128, NCH], DT, tag="p_t")
            nc.tensor.transpose(p_hc[:], h_rows_full[:], ident32[:])
            h_cols = sb.tile([128, NCH], DTR, tag="h_cols")
            nc.vector.tensor_copy(h_cols[:], p_hc[:])

            # ---- out Linear (vocab-sharded): logits = h_new @ out_W.T + b ----
            p_l = ps.tile([1, SH], DT, tag="p_xl")
            for t in range(2):
                wt = wpool.tile([128, 16, SH], DTR, tag="wt")
                nc.sync.dma_start(
                    wt[:], out_wt_i[2048 * t:2048 * (t + 1), :]
                    .rearrange("(c p) n -> p c n", p=128))
                _emit_mv(nc, p_l[:], wt, 2048, t,
                         lambda c: h_cols[:, c:c + 1], SH, 32, 16 * t)
            l_row = sb.tile([1, SH], DT, tag="l_row")
            nc.vector.tensor_add(l_row[:], p_l[:], small_rows["out_b"][:])

            # ---- AllGather logits ----
            for i in range(4):
                nc.sync.dma_start(ag_l_in[i:i + 1, :], l_row[0:1, 128 * i:128 * (i + 1)])
            nc.gpsimd.collective_compute(
                "AllGather", ALU.bypass, replica_groups=RG,
                ins=[ag_l_in[:].opt()], outs=[ag_l_out[:].opt()])
            lg = sb.tile([NCH, 128], DT, tag="lg")
            nc.sync.dma_start(lg[:], ag_l_out[:])

            # ---- log_softmax over all 4096, [32, 128] layout ----
            pm = sb.tile([NCH, 1], DT, tag="pm")
            nc.vector.reduce_max(pm[:], lg[:], axis=mybir.AxisListType.X)
            p_pmt = ps.tile([1, NCH], DT, tag="p_t")
            nc.tensor.transpose(p_pmt[:], pm[:], ident32[:])
            pmt = sb.tile([1, NCH], DT, tag="pmt")
            nc.vector.tensor_copy(pmt[:], p_pmt[:])
            gmax = sb.tile([1, 1], DT, tag="gmax")
            nc.vector.reduce_max(gmax[:], pmt[:], axis=mybir.AxisListType.X)
            ngmax = sb.tile([1, 1], DT, tag="ngmax")
            nc.vector.tensor_scalar_mul(ngmax[:], gmax[:], -1.0)
            p_nb = ps.tile([NCH, 1], DT, tag="p_t")
            nc.tensor.matmul(p_nb[:], ones32[:], ngmax[:], start=True, stop=True)
            nmax_col = sb.tile([NCH, 1], DT, tag="nmax_col")
            nc.vector.tensor_copy(nmax_col[:], p_nb[:])
            exp_t = sb.tile([NCH, 128], DT, tag="exp_t")
            sum_col = sb.tile([NCH, 1], DT, tag="sum_col")
            nc.scalar.activation(exp_t[:], lg[:], AF.Exp, bias=nmax_col[:],
                                 accum_out=sum_col[:])
            p_st = ps.tile([1, NCH], DT, tag="p_t")
            nc.tensor.transpose(p_st[:], sum_col[:], ident32[:])
            st = sb.tile([1, NCH], DT, tag="st")
            nc.vector.tensor_copy(st[:], p_st[:])
            gsum = sb.tile([1, 1], DT, tag="gsum")
            nc.vector.reduce_sum(gsum[:], st[:], axis=mybir.AxisListType.X)
            lse = sb.tile([1, 1], DT, tag="lse")
            nc.scalar.activation(lse[:], gsum[:], AF.Ln)
            nc.vector.tensor_add(lse[:], lse[:], gmax[:])
            nlse = sb.tile([1, 1], DT, tag="nlse")
            nc.vector.tensor_scalar_mul(nlse[:], lse[:], -1.0)
            p_nl = ps.tile([NCH, 1], DT, tag="p_t")
            nc.tensor.matmul(p_nl[:], ones32[:], nlse[:], start=True, stop=True)
            nlse_col = sb.tile([NCH, 1], DT, tag="nlse_col")
            nc.vector.tensor_copy(nlse_col[:], p_nl[:])
            out_t = sb.tile([NCH, 128], DT, tag="out_t")
            nc.scalar.activation(out_t[:], lg[:], AF.Identity, bias=nlse_col[:])
            nc.sync.dma_start(out_ext[:], out_t[:])

    nc.finalize()
    return nc


def _get_nc():
    global _NC_CACHE
    if _NC_CACHE is None:
        _NC_CACHE = _build_nc()
    return _NC_CACHE


def _prepare_in_maps(input_ids, hidden, e_output_seq, emb_W, att_W, att_b,
                     comb_W, comb_b, gru_W_ih, gru_W_hh, gru_b_ih, gru_b_hh,
                     out_W, out_b):
    f32 = np.float32
    emb_row = np.ascontiguousarray(emb_W[int(input_ids[0])], dtype=f32)   # [O]
    h0 = np.ascontiguousarray(hidden[0, 0], dtype=f32)                    # [H]
    emb_cols = np.ascontiguousarray(emb_row.reshape(NCH, 128).T)
    h0_cols = np.ascontiguousarray(h0.reshape(NCH, 128).T)
    att_wt = np.zeros((2 * H, 16), f32)
    att_wt[:, :MAX_LEN] = np.asarray(att_W, f32).T
    att_b16 = np.zeros((1, 16), f32)
    att_b16[0, :MAX_LEN] = np.asarray(att_b, f32)
    e_seq = np.ascontiguousarray(e_output_seq, dtype=f32)
    W_ih = np.asarray(gru_W_ih, f32)
    W_hh = np.asarray(gru_W_hh, f32)
    b_rz_full = (np.asarray(gru_b_ih, f32) + np.asarray(gru_b_hh, f32))

    in_maps = []
    for r in range(N_CORES):
        sl = slice(SH * r, SH * (r + 1))
        r_rows = slice(SH * r, SH * (r + 1))
        z_rows = slice(H + SH * r, H + SH * (r + 1))
        n_rows = slice(2 * H + SH * r, 2 * H + SH * (r + 1))
        rz_wt = np.empty((2 * H, 2 * SH), f32)
        rz_wt[:H, :SH] = W_ih[r_rows].T
        rz_wt[:H, SH:] = W_ih[z_rows].T
        rz_wt[H:, :SH] = W_hh[r_rows].T
        rz_wt[H:, SH:] = W_hh[z_rows].T
        rz_b = np.concatenate([b_rz_full[r_rows], b_rz_full[z_rows]])[None, :]
        in_maps.append({
            "emb_cols": emb_cols,
            "h0_cols": h0_cols,
            "h0_shard": np.ascontiguousarray(h0[sl])[None, :],
            "e_seq": e_seq,
            "att_wt": att_wt,
            "att_b": att_b16,
            "comb_wt": np.ascontiguousarray(np.asarray(comb_W, f32)[sl].T),
            "comb_b": np.asarray(comb_b, f32)[sl][None, :],
            "rz_wt": rz_wt,
            "rz_b": np.ascontiguousarray(rz_b),
            "in_wt": np.ascontiguousarray(W_ih[n_rows].T),
            "in_b": np.asarray(gru_b_ih, f32)[n_rows][None, :],
            "hn_wt": np.ascontiguousarray(W_hh[n_rows].T),
            "hn_b": np.asarray(gru_b_hh, f32)[n_rows][None, :],
            "out_wt": np.ascontiguousarray(np.asarray(out_W, f32)[sl].T),
            "out_b": np.asarray(out_b, f32)[sl][None, :],
        })
    return in_maps


def kernel(input_ids, hidden, e_output, e_output_seq, emb_W, att_W, att_b,
           comb_W, comb_b, gru_W_ih, gru_W_hh, gru_b_ih, gru_b_hh, out_W,
           out_b):
    nc = _get_nc()
    in_maps = _prepare_in_maps(
        np.asarray(input_ids), np.asarray(hidden), np.asarray(e_output_seq),
        np.asarray(emb_W), np.asarray(att_W), np.asarray(att_b),
        np.asarray(comb_W), np.asarray(comb_b), np.asarray(gru_W_ih),
        np.asarray(gru_W_hh), np.asarray(gru_b_ih), np.asarray(gru_b_hh),
        np.asarray(out_W), np.asarray(out_b))
    res = run_bass_kernel_spmd(nc, in_maps, core_ids=list(range(N_CORES)))
    r0 = res.results[0]
    out = r0["out_sm"].reshape(1, O)
    h_new = r0["h_new_full"].reshape(1, 1, H)
    att_w = r0["att_w"].reshape(1, MAX_LEN)
    return out, h_new, att_w


def _exec_setup(in_maps):
    """Build a reusable jitted executable with device-resident inputs."""
    import jax
    from jax.sharding import Mesh, NamedSharding, PartitionSpec
    from jax.experimental.shard_map import shard_map
    from concourse import bass2jax, mybir as _mybir

    nc = _get_nc()
    bass2jax.install_neuronx_cc_hook()
    partition_name = nc.partition_id_tensor.name if nc.partition_id_tensor else None
    in_names, out_names, out_avals, zero_outs = [], [], [], []
    for alloc in nc.m.functions[0].allocations:
        if not isinstance(alloc, _mybir.MemoryLocationSet):
            continue
        name = alloc.memorylocations[0].name
        if alloc.kind == "ExternalInput":
            if name != partition_name:
                in_names.append(name)
        elif alloc.kind == "ExternalOutput":
            shape = tuple(alloc.tensor_shape)
            dtype = _mybir.dt.np(alloc.dtype)
            out_names.append(name)
            out_avals.append(jax.core.ShapedArray(shape, dtype))
            zero_outs.append(np.zeros(shape, dtype))
    n_params = len(in_names)
    all_in_names = list(in_names) + list(out_names)
    if partition_name is not None:
        all_in_names.append(partition_name)

    def _body(*args):
        operands = list(args)
        if partition_name is not None:
            operands.append(bass2jax.partition_id_tensor())
        outs = bass2jax._bass_exec_p.bind(
            *operands,
            out_avals=tuple(out_avals),
            in_names=tuple(all_in_names),
            out_names=tuple(out_names),
            lowering_input_output_aliases=(),
            sim_require_finite=True,
            sim_require_nnan=True,
            nc=nc,
        )
        return tuple(outs)

    devices = jax.devices()[:N_CORES]
    mesh = Mesh(np.asarray(devices), ("core",))
    n_all = n_params + len(zero_outs)
    sharded = jax.jit(
        shard_map(_body, mesh=mesh,
                  in_specs=(PartitionSpec("core"),) * n_all,
                  out_specs=(PartitionSpec("core"),) * len(out_names),
                  check_rep=False),
        keep_unused=True)
    sh = NamedSharding(mesh, PartitionSpec("core"))
    concat_in = [
        np.concatenate([np.asarray(in_maps[c][n]) for c in range(N_CORES)], axis=0)
        for n in in_names
    ]
    concat_zeros = [np.zeros((N_CORES * z.shape[0], *z.shape[1:]), z.dtype)
                    for z in zero_outs]
    dev_in = [jax.device_put(a, sh) for a in concat_in + concat_zeros]
    return sharded, dev_in, out_names


def bench(inputs, iters=12):
    """Estimate per-execution device time (ns) via async-chain slope."""
    import time as _time
    import jax
    in_maps = _prepare_in_maps(
        np.asarray(inputs["input_ids"]), np.asarray(inputs["hidden"]),
        np.asarray(inputs["e_output_seq"]), np.asarray(inputs["emb_W"]),
        np.asarray(inputs["att_W"]), np.asarray(inputs["att_b"]),
        np.asarray(inputs["comb_W"]), np.asarray(inputs["comb_b"]),
        np.asarray(inputs["gru_W_ih"]), np.asarray(inputs["gru_W_hh"]),
        np.asarray(inputs["gru_b_ih"]), np.asarray(inputs["gru_b_hh"]),
        np.asarray(inputs["out_W"]), np.asarray(inputs["out_b"]))
    sharded, dev_in, _ = _exec_setup(in_maps)

    def chain(k):
        r = None
        t0 = _time.perf_counter()
        for _ in range(k):
            r = sharded(*dev_in)
        jax.block_until_ready(r)
        return _time.perf_counter() - t0

    chain(3)  # warm up compile + caches
    k1, k2 = 8, 8 + max(4, iters)
    t1 = min(chain(k1) for _ in range(3))
    t2 = min(chain(k2) for _ in range(3))
    per_exec = (t2 - t1) / (k2 - k1)
    print(f"[bench] chain({k1})={t1*1e3:.2f}ms chain({k2})={t2*1e3:.2f}ms "
          f"-> per-exec {per_exec*1e6:.1f}us")
    return per_exec * 1e9


if __name__ == "__main__":
    rng = np.random.default_rng(0)
    s = 0.02
    inp = dict(
        input_ids=np.array([7], np.int64),
        hidden=rng.normal(size=(1, 1, H)).astype(np.float32),
        e_output=rng.normal(size=(1, H)).astype(np.float32),
        e_output_seq=rng.normal(size=(MAX_LEN, H)).astype(np.float32),
        emb_W=(rng.normal(size=(H, O)) * s).astype(np.float32),
        att_W=(rng.normal(size=(MAX_LEN, H + O)) * s).astype(np.float32),
        att_b=np.zeros(MAX_LEN, np.float32),
        comb_W=(rng.normal(size=(H, H + O)) * s).astype(np.float32),
        comb_b=np.zeros(H, np.float32),
        gru_W_ih=(rng.normal(size=(3 * H, H)) * s).astype(np.float32),
        gru_W_hh=(rng.normal(size=(3 * H, H)) * s).astype(np.float32),
        gru_b_ih=np.zeros(3 * H, np.float32),
        gru_b_hh=np.zeros(3 * H, np.float32),
        out_W=(rng.normal(size=(O, H)) * s).astype(np.float32),
        out_b=np.zeros(O, np.float32),
    )
    out, h_new, att_w = kernel(**inp)

    # numpy reference
    def np_ref(i):
        emb = i["emb_W"][int(i["input_ids"][0])][None, :]
        h0 = i["hidden"][0]
        al = np.concatenate([emb, h0], 1) @ i["att_W"].T + i["att_b"]
        aw = np.exp(al - al.max())
        aw = aw / aw.sum()
        aa = aw @ i["e_output_seq"]
        comb = np.concatenate([emb, aa], 1) @ i["comb_W"].T + i["comb_b"]
        x = np.maximum(comb, 0)
        gi = x @ i["gru_W_ih"].T + i["gru_b_ih"]
        gh = h0 @ i["gru_W_hh"].T + i["gru_b_hh"]
        ir, iz, inn = np.split(gi, 3, 1)
        hr, hz, hn = np.split(gh, 3, 1)
        r = 1 / (1 + np.exp(-(ir + hr)))
        z = 1 / (1 + np.exp(-(iz + hz)))
        n = np.tanh(inn + r * hn)
        hnew = (1 - z) * n + z * h0
        lo = hnew @ i["out_W"].T + i["out_b"]
        ls = lo - (np.log(np.exp(lo - lo.max()).sum()) + lo.max())
        return ls, hnew[None], aw

    eo, eh, ea = np_ref({k: np.asarray(v, np.float64) for k, v in inp.items()})
    for name, got, exp in (("out", out, eo), ("h_new", h_new, eh), ("att_w", att_w, ea)):
        err = np.abs(got - exp).max() / (np.abs(exp).max() + 1e-30)
        print(f"{name}: rel err {err:.3e}")


# revision 8
# speedup vs baseline: 1.2800x; 1.2800x over previous
"""AttentionDecoder step (batch=1) on 8 TRN2 NeuronCores, tensor-parallel.

Layers: emb lookup -> attention over 15 encoder positions -> combine Linear
(relu) -> GRU cell -> out Linear + log_softmax.

Sharding (8 cores): column-parallel on every big Linear. Core r owns rows
[512r, 512r+512) of comb_W, of each GRU gate (r/z/n), and of out_W. The tiny
[1, 512] per-core activations (x, h_new, logits) are all-gathered on device.

Memory regime: ~604MB of f32 weights stream once per call (~76MB/core).
The host pre-transposes weights to [K, M] layout so the TensorEngine can
stream them as the moving operand against a stationary activation-column
chunk ([128, 1]); psum accumulates [1, 512] rows over K chunks.

Activation layout convention: a length-N vector v is held either as a
"row" [1, N] on one partition, or as "cols" [128, N/128] with
cols[p, c] = v[128c + p]. Host-provided vectors (emb, h0) arrive as cols;
device-computed rows are converted to cols via AllGather + PE transpose.
"""

import os
import sys

sys.path.insert(0, "/opt/trn_rl_repo")

import numpy as np

import concourse.bass as bass
import concourse.mybir as mybir
import concourse.tile as tile
from concourse import bacc
from concourse.bass_utils import run_bass_kernel_spmd
from concourse.masks import make_identity

H = 4096
O = 4096
MAX_LEN = 15
N_CORES = 8
SH = H // N_CORES          # 512, per-core shard of any 4096-dim output
NCH = H // 128             # 32 chunks of 128 in a 4096 vector
DT = mybir.dt.float32
DTR = mybir.dt.float32r
AF = mybir.ActivationFunctionType
ALU = mybir.AluOpType
RG = [list(range(N_CORES))]

_NC_CACHE = None


def _emit_mv(nc, psum_ap, wtile, rows_per_tile, t, lhsT_of_chunk, width, k_chunks_total,
             tile_chunk0):
    """Matmuls for one weight tile: psum[1, width] += lhsT_c.T @ w[:, slice]."""
    cpt = rows_per_tile // 128
    for cl in range(cpt):
        c = tile_chunk0 + cl
        nc.tensor.matmul(
            psum_ap,
            lhsT_of_chunk(c),
            wtile[:, cl, 0:width],
            start=(c == 0),
            stop=(c == k_chunks_total - 1),
        )


def _build_nc():
    nc = bacc.Bacc("TRN2", target_bir_lowering=False, debug=False,
                   num_devices=N_CORES)

    # ---- I/O ----
    emb_cols_i = nc.dram_tensor("emb_cols", [128, NCH], DTR, kind="ExternalInput")
    h0_cols_i = nc.dram_tensor("h0_cols", [128, NCH], DTR, kind="ExternalInput")
    h0_shard_i = nc.dram_tensor("h0_shard", [1, SH], DT, kind="ExternalInput")
    e_seq_i = nc.dram_tensor("e_seq", [MAX_LEN, H], DT, kind="ExternalInput")
    att_wt_i = nc.dram_tensor("att_wt", [2 * H, 16], DTR, kind="ExternalInput")
    att_b_i = nc.dram_tensor("att_b", [1, 16], DT, kind="ExternalInput")
    comb_wt_i = nc.dram_tensor("comb_wt", [2 * H, SH], DTR, kind="ExternalInput")
    comb_b_i = nc.dram_tensor("comb_b", [1, SH], DT, kind="ExternalInput")
    rz_wt_i = nc.dram_tensor("rz_wt", [2 * H, 2 * SH], DTR, kind="ExternalInput")
    rz_b_i = nc.dram_tensor("rz_b", [1, 2 * SH], DT, kind="ExternalInput")
    in_wt_i = nc.dram_tensor("in_wt", [H, SH], DTR, kind="ExternalInput")
    in_b_i = nc.dram_tensor("in_b", [1, SH], DT, kind="ExternalInput")
    hn_wt_i = nc.dram_tensor("hn_wt", [H, SH], DTR, kind="ExternalInput")
    hn_b_i = nc.dram_tensor("hn_b", [1, SH], DT, kind="ExternalInput")
    out_wt_i = nc.dram_tensor("out_wt", [SH, H], DTR, kind="ExternalInput")
    out_b_i = nc.dram_tensor("out_b", [NCH, 128], DT, kind="ExternalInput")

    out_ext = nc.dram_tensor("out_sm", [NCH, 128], DT, kind="ExternalOutput")
    hnew_ext = nc.dram_tensor("h_new_shard", [1, SH], DT, kind="ExternalOutput")
    attw_ext = nc.dram_tensor("att_w", [1, MAX_LEN], DT, kind="ExternalOutput")

    with tile.TileContext(nc) as tc:
        with (
            tc.tile_pool(name="wpool", bufs=4) as wpool,
            tc.tile_pool(name="sb", bufs=1) as sb,
            tc.tile_pool(name="ps", bufs=1, space="PSUM") as ps,
            tc.tile_pool(name="ps2", bufs=2, space="PSUM") as ps2,
            tc.tile_pool(name="dram", bufs=1, space="DRAM") as dram,
        ):
            # ---- small resident loads ----
            e_seq = sb.tile([MAX_LEN, H], DT, tag="e_seq")
            nc.sync.dma_start(e_seq[:], e_seq_i[:])
            att_sb = sb.tile([128, 64 * 16], DTR, tag="att_sb")
            for c in range(64):
                nc.sync.dma_start(att_sb[:, 16 * c:16 * (c + 1)],
                                  att_wt_i[128 * c:128 * (c + 1), :])
            emb_cols = sb.tile([128, NCH], DTR, tag="emb_cols")
            nc.sync.dma_start(emb_cols[:], emb_cols_i[:])
            h0_cols = sb.tile([128, NCH], DTR, tag="h0_cols")
            nc.sync.dma_start(h0_cols[:], h0_cols_i[:])
            small_rows = {}
            for name, t_in, width in (
                ("att_b", att_b_i, 16), ("comb_b", comb_b_i, SH),
                ("rz_b", rz_b_i, 2 * SH), ("in_b", in_b_i, SH),
                ("hn_b", hn_b_i, SH),
                ("h0_shard", h0_shard_i, SH),
            ):
                tl = sb.tile([1, width], DT, tag=name)
                nc.sync.dma_start(tl[:], t_in[:])
                small_rows[name] = tl

            outb_rows = sb.tile([NCH, 128], DT, tag="outb_rows")
            nc.sync.dma_start(outb_rows[:], out_b_i[:])

            ident32 = sb.tile([32, 32], DT, tag="ident32")
            make_identity(nc, ident32[:])
            ident1 = sb.tile([1, 1], DT, tag="ident1")
            nc.gpsimd.memset(ident1[:], 1.0)
            ones32 = sb.tile([1, 32], DT, tag="ones32")
            nc.gpsimd.memset(ones32[:], 1.0)

            # dram bounce buffers for collectives
            ag_x_in = dram.tile([4, 128], DT, tag="ag_x_in")
            ag_x_out = dram.tile([NCH, 128], DT, tag="ag_x_out")
            ar_l_in = dram.tile([NCH, 128], DT, tag="ar_l_in")
            ar_l_out = dram.tile([NCH, 128], DT, tag="ar_l_out")

            def eh_chunk(c):
                return emb_cols[:, c - 0:c + 1] if c < NCH else h0_cols[:, c - NCH:c - NCH + 1]

            # ---- attention logits: [1,16] = sum_c eh_c.T @ att_sb_c ----
            p_attl = ps.tile([1, 16], DT, tag="p_t")
            for c in range(64):
                nc.tensor.matmul(p_attl[:], eh_chunk(c), att_sb[:, 16 * c:16 * (c + 1)],
                                 start=(c == 0), stop=(c == 63))

            # softmax over the first 15 entries (row layout, 1 partition)
            attl_row = sb.tile([1, MAX_LEN], DT, tag="attl_row")
            nc.vector.tensor_add(attl_row[:], p_attl[:, 0:MAX_LEN],
                                 small_rows["att_b"][:, 0:MAX_LEN])
            mx = sb.tile([1, 1], DT, tag="attl_mx")
            nc.vector.reduce_max(mx[:], attl_row[:], axis=mybir.AxisListType.X)
            nmx = sb.tile([1, 1], DT, tag="attl_nmx")
            nc.vector.tensor_scalar_mul(nmx[:], mx[:], -1.0)
            expr = sb.tile([1, MAX_LEN], DT, tag="attl_exp")
            sume = sb.tile([1, 1], DT, tag="attl_sum")
            nc.scalar.activation(expr[:], attl_row[:], AF.Exp, bias=nmx[:],
                                 accum_out=sume[:])
            rsum = sb.tile([1, 1], DT, tag="attl_rsum")
            nc.vector.reciprocal(rsum[:], sume[:])
            attw_row = sb.tile([1, MAX_LEN], DT, tag="attw_row")
            nc.vector.tensor_scalar_mul(attw_row[:], expr[:], rsum[:])
            nc.sync.dma_start(attw_ext[:], attw_row[:])

            # transpose att weights row -> column [15, 1]
            p_awc = ps.tile([MAX_LEN, 1], DT, tag="p_t")
            nc.tensor.transpose(p_awc[:], attw_row[:], ident1[:])
            aw_col = sb.tile([MAX_LEN, 1], DT, tag="aw_col")
            nc.vector.tensor_copy(aw_col[:], p_awc[:])

            # att_applied as columns: [128, 32], chunk j = e_seq[:, 128j:].T @ aw
            p_attc = ps.tile([128, NCH], DT, tag="p_t")
            for j in range(NCH):
                nc.tensor.matmul(p_attc[:, j:j + 1], e_seq[:, 128 * j:128 * (j + 1)],
                                 aw_col[:], start=True, stop=True)
            att_cols = sb.tile([128, NCH], DTR, tag="att_cols")
            nc.vector.tensor_copy(att_cols[:], p_attc[:])

            def comb_chunk(c):
                return emb_cols[:, c:c + 1] if c < NCH else att_cols[:, c - NCH:c - NCH + 1]

            # ---- combine Linear: x = relu(combined @ comb_W.T + b), sharded ----
            p_x = ps.tile([1, SH], DT, tag="p_xl")
            for t in range(4):
                wt = wpool.tile([128, 16, SH], DTR, tag="wt")
                nc.sync.dma_start(
                    wt[:], comb_wt_i[2048 * t:2048 * (t + 1), :]
                    .rearrange("(c p) n -> p c n", p=128))
                _emit_mv(nc, p_x[:], wt, 2048, t, comb_chunk, SH, 64, 16 * t)
            x_row = sb.tile([1, SH], DT, tag="x_row")
            nc.vector.tensor_add(x_row[:], p_x[:], small_rows["comb_b"][:])
            nc.vector.tensor_scalar_max(x_row[:], x_row[:], 0.0)

            # ---- AllGather x -> x_cols [128, 32] ----
            for i in range(4):
                nc.sync.dma_start(ag_x_in[i:i + 1, :], x_row[0:1, 128 * i:128 * (i + 1)])
            nc.gpsimd.collective_compute(
                "AllGather", ALU.bypass, replica_groups=RG,
                ins=[ag_x_in[:].opt()], outs=[ag_x_out[:].opt()])
            x_rows_full = sb.tile([NCH, 128], DT, tag="x_rows_full")
            nc.sync.dma_start(x_rows_full[:], ag_x_out[:])
            p_xc = ps.tile([128, NCH], DT, tag="p_t")
            nc.tensor.transpose(p_xc[:], x_rows_full[:], ident32[:])
            x_cols = sb.tile([128, NCH], DTR, tag="x_cols")
            i_xcols_copy = nc.vector.tensor_copy(x_cols[:], p_xc[:])

            def xh_chunk(c):
                return x_cols[:, c:c + 1] if c < NCH else h0_cols[:, c - NCH:c - NCH + 1]

            # ---- GRU hh n-gate first (only needs h0) ----
            p_hn = ps.tile([1, SH], DT, tag="p_hn")
            for t in range(2):
                wt = wpool.tile([128, 16, SH], DTR, tag="wt")
                nc.sync.dma_start(
                    wt[:], hn_wt_i[2048 * t:2048 * (t + 1), :]
                    .rearrange("(c p) n -> p c n", p=128))
                _emit_mv(nc, p_hn[:], wt, 2048, t,
                         lambda c: h0_cols[:, c:c + 1], SH, 32, 16 * t)

            # ---- GRU r,z gates (stacked [x; h0] contraction) ----
            p_r = ps.tile([1, SH], DT, tag="p_r")
            p_z = ps.tile([1, SH], DT, tag="p_z")
            for t in range(8):
                wt = wpool.tile([128, 8, 2 * SH], DTR, tag="wt")
                nc.sync.dma_start(
                    wt[:], rz_wt_i[1024 * t:1024 * (t + 1), :]
                    .rearrange("(c p) n -> p c n", p=128))
                for cl in range(8):
                    c = 8 * t + cl
                    nc.tensor.matmul(p_r[:], xh_chunk(c),
                                     wt[:, cl, 0:SH],
                                     start=(c == 0), stop=(c == 63))
                    nc.tensor.matmul(p_z[:], xh_chunk(c),
                                     wt[:, cl, SH:2 * SH],
                                     start=(c == 0), stop=(c == 63))

            # ---- GRU ih n-gate (needs x) ----
            p_in = ps.tile([1, SH], DT, tag="p_in")
            for t in range(2):
                wt = wpool.tile([128, 16, SH], DTR, tag="wt")
                nc.sync.dma_start(
                    wt[:], in_wt_i[2048 * t:2048 * (t + 1), :]
                    .rearrange("(c p) n -> p c n", p=128))
                _emit_mv(nc, p_in[:], wt, 2048, t,
                         lambda c: x_cols[:, c:c + 1], SH, 32, 16 * t)

            # ---- GRU cell elementwise (row layout, 1 partition) ----
            r_row = sb.tile([1, SH], DT, tag="r_row")
            i_radd = nc.vector.tensor_add(r_row[:], p_r[:], small_rows["rz_b"][:, 0:SH])
            tile.add_dep_helper(i_radd.ins, i_xcols_copy.ins, sync=False,
                                reason="keep gru DVE ops behind x_cols copy")
            nc.scalar.activation(r_row[:], r_row[:], AF.Sigmoid)
            z_row = sb.tile([1, SH], DT, tag="z_row")
            nc.vector.tensor_add(z_row[:], p_z[:], small_rows["rz_b"][:, SH:2 * SH])
            nc.scalar.activation(z_row[:], z_row[:], AF.Sigmoid)
            hn_row = sb.tile([1, SH], DT, tag="hn_row")
            nc.vector.tensor_add(hn_row[:], p_hn[:], small_rows["hn_b"][:])
            rhn = sb.tile([1, SH], DT, tag="rhn")
            nc.vector.tensor_mul(rhn[:], r_row[:], hn_row[:])
            n_row = sb.tile([1, SH], DT, tag="n_row")
            nc.vector.tensor_add(n_row[:], p_in[:], small_rows["in_b"][:])
            nc.vector.tensor_add(n_row[:], n_row[:], rhn[:])
            nc.scalar.activation(n_row[:], n_row[:], AF.Tanh)
            # h_new = n + z * (h0 - n)
            d_row = sb.tile([1, SH], DT, tag="d_row")
            nc.vector.tensor_sub(d_row[:], small_rows["h0_shard"][:], n_row[:])
            nc.vector.tensor_mul(d_row[:], z_row[:], d_row[:])
            hnew_row = sb.tile([1, SH], DT, tag="hnew_row")
            nc.vector.tensor_add(hnew_row[:], n_row[:], d_row[:])

            # ---- h_new shard out + transpose to cols [128, 4] ----
            nc.sync.dma_start(hnew_ext[:], hnew_row[:])
            p_hsc = ps.tile([128, 4], DT, tag="p_t")
            for i in range(4):
                nc.tensor.transpose(p_hsc[:, i:i + 1],
                                    hnew_row[0:1, 128 * i:128 * (i + 1)], ident1[:])
            h_sh_cols = sb.tile([128, 4], DTR, tag="h_sh_cols")
            nc.vector.tensor_copy(h_sh_cols[:], p_hsc[:])

            # ---- out Linear (row-parallel): partial logits [1, 4096] ----
            out_tiles = []
            for t in range(2):
                wt = wpool.tile([128, 2, H], DTR, tag="wt")
                nc.sync.dma_start(
                    wt[:], out_wt_i[256 * t:256 * (t + 1), :]
                    .rearrange("(c p) n -> p c n", p=128))
                out_tiles.append(wt)
            l_full = sb.tile([1, H], DT, tag="l_full")
            for b in range(8):
                po = ps2.tile([1, SH], DT, tag="p_o")
                for k in range(4):
                    nc.tensor.matmul(
                        po[:], h_sh_cols[:, k:k + 1],
                        out_tiles[k // 2][:, k % 2, SH * b:SH * (b + 1)],
                        start=(k == 0), stop=(k == 3))
                nc.vector.tensor_copy(l_full[:, SH * b:SH * (b + 1)], po[:])

            # ---- AllReduce partial logits ----
            for c in range(NCH):
                nc.sync.dma_start(ar_l_in[c:c + 1, :], l_full[0:1, 128 * c:128 * (c + 1)])
            nc.gpsimd.collective_compute(
                "AllReduce", ALU.add, replica_groups=RG,
                ins=[ar_l_in[:].opt()], outs=[ar_l_out[:].opt()])
            lg = sb.tile([NCH, 128], DT, tag="lg")
            nc.sync.dma_start(lg[:], ar_l_out[:])
            nc.vector.tensor_add(lg[:], lg[:], outb_rows[:])

            # ---- log_softmax over all 4096, [32, 128] layout ----
            pm = sb.tile([NCH, 1], DT, tag="pm")
            nc.vector.reduce_max(pm[:], lg[:], axis=mybir.AxisListType.X)
            p_pmt = ps.tile([1, NCH], DT, tag="p_t")
            nc.tensor.transpose(p_pmt[:], pm[:], ident32[:])
            pmt = sb.tile([1, NCH], DT, tag="pmt")
            nc.vector.tensor_copy(pmt[:], p_pmt[:])
            gmax = sb.tile([1, 1], DT, tag="gmax")
            nc.vector.reduce_max(gmax[:], pmt[:], axis=mybir.AxisListType.X)
            ngmax = sb.tile([1, 1], DT, tag="ngmax")
            nc.vector.tensor_scalar_mul(ngmax[:], gmax[:], -1.0)
            p_nb = ps.tile([NCH, 1], DT, tag="p_t")
            nc.tensor.matmul(p_nb[:], ones32[:], ngmax[:], start=True, stop=True)
            nmax_col = sb.tile([NCH, 1], DT, tag="nmax_col")
            nc.vector.tensor_copy(nmax_col[:], p_nb[:])
            exp_t = sb.tile([NCH, 128], DT, tag="exp_t")
            sum_col = sb.tile([NCH, 1], DT, tag="sum_col")
            nc.scalar.activation(exp_t[:], lg[:], AF.Exp, bias=nmax_col[:],
                                 accum_out=sum_col[:])
            p_st = ps.tile([1, NCH], DT, tag="p_t")
            nc.tensor.transpose(p_st[:], sum_col[:], ident32[:])
            st = sb.tile([1, NCH], DT, tag="st")
            nc.vector.tensor_copy(st[:], p_st[:])
            gsum = sb.tile([1, 1], DT, tag="gsum")
            nc.vector.reduce_sum(gsum[:], st[:], axis=mybir.AxisListType.X)
            lse = sb.tile([1, 1], DT, tag="lse")
            nc.scalar.activation(lse[:], gsum[:], AF.Ln)
            nc.vector.tensor_add(lse[:], lse[:], gmax[:])
            nlse = sb.tile([1, 1], DT, tag="nlse")
            nc.vector.tensor_scalar_mul(nlse[:], lse[:], -1.0)
            p_nl = ps.tile([NCH, 1], DT, tag="p_t")
            nc.tensor.matmul(p_nl[:], ones32[:], nlse[:], start=True, stop=True)
            nlse_col = sb.tile([NCH, 1], DT, tag="nlse_col")
            nc.vector.tensor_copy(nlse_col[:], p_nl[:])
            out_t = sb.tile([NCH, 128], DT, tag="out_t")
            nc.scalar.activation(out_t[:], lg[:], AF.Identity, bias=nlse_col[:])
            nc.sync.dma_start(out_ext[:], out_t[:])

    nc.finalize()
    return nc


def _get_nc():
    global _NC_CACHE
    if _NC_CACHE is None:
        _NC_CACHE = _build_nc()
    return _NC_CACHE


def _prepare_in_maps(input_ids, hidden, e_output_seq, emb_W, att_W, att_b,
                     comb_W, comb_b, gru_W_ih, gru_W_hh, gru_b_ih, gru_b_hh,
                     out_W, out_b):
    f32 = np.float32
    emb_row = np.ascontiguousarray(emb_W[int(input_ids[0])], dtype=f32)   # [O]
    h0 = np.ascontiguousarray(hidden[0, 0], dtype=f32)                    # [H]
    emb_cols = np.ascontiguousarray(emb_row.reshape(NCH, 128).T)
    h0_cols = np.ascontiguousarray(h0.reshape(NCH, 128).T)
    att_wt = np.zeros((2 * H, 16), f32)
    att_wt[:, :MAX_LEN] = np.asarray(att_W, f32).T
    att_b16 = np.zeros((1, 16), f32)
    att_b16[0, :MAX_LEN] = np.asarray(att_b, f32)
    e_seq = np.ascontiguousarray(e_output_seq, dtype=f32)
    W_ih = np.asarray(gru_W_ih, f32)
    W_hh = np.asarray(gru_W_hh, f32)
    b_rz_full = (np.asarray(gru_b_ih, f32) + np.asarray(gru_b_hh, f32))

    in_maps = []
    for r in range(N_CORES):
        sl = slice(SH * r, SH * (r + 1))
        r_rows = slice(SH * r, SH * (r + 1))
        z_rows = slice(H + SH * r, H + SH * (r + 1))
        n_rows = slice(2 * H + SH * r, 2 * H + SH * (r + 1))
        rz_wt = np.empty((2 * H, 2 * SH), f32)
        rz_wt[:H, :SH] = W_ih[r_rows].T
        rz_wt[:H, SH:] = W_ih[z_rows].T
        rz_wt[H:, :SH] = W_hh[r_rows].T
        rz_wt[H:, SH:] = W_hh[z_rows].T
        rz_b = np.concatenate([b_rz_full[r_rows], b_rz_full[z_rows]])[None, :]
        in_maps.append({
            "emb_cols": emb_cols,
            "h0_cols": h0_cols,
            "h0_shard": np.ascontiguousarray(h0[sl])[None, :],
            "e_seq": e_seq,
            "att_wt": att_wt,
            "att_b": att_b16,
            "comb_wt": np.ascontiguousarray(np.asarray(comb_W, f32)[sl].T),
            "comb_b": np.asarray(comb_b, f32)[sl][None, :],
            "rz_wt": rz_wt,
            "rz_b": np.ascontiguousarray(rz_b),
            "in_wt": np.ascontiguousarray(W_ih[n_rows].T),
            "in_b": np.asarray(gru_b_ih, f32)[n_rows][None, :],
            "hn_wt": np.ascontiguousarray(W_hh[n_rows].T),
            "hn_b": np.asarray(gru_b_hh, f32)[n_rows][None, :],
            "out_wt": np.ascontiguousarray(np.asarray(out_W, f32)[:, sl].T),
            "out_b": np.ascontiguousarray(np.asarray(out_b, f32).reshape(NCH, 128)),
        })
    return in_maps


def kernel(input_ids, hidden, e_output, e_output_seq, emb_W, att_W, att_b,
           comb_W, comb_b, gru_W_ih, gru_W_hh, gru_b_ih, gru_b_hh, out_W,
           out_b):
    nc = _get_nc()
    in_maps = _prepare_in_maps(
        np.asarray(input_ids), np.asarray(hidden), np.asarray(e_output_seq),
        np.asarray(emb_W), np.asarray(att_W), np.asarray(att_b),
        np.asarray(comb_W), np.asarray(comb_b), np.asarray(gru_W_ih),
        np.asarray(gru_W_hh), np.asarray(gru_b_ih), np.asarray(gru_b_hh),
        np.asarray(out_W), np.asarray(out_b))
    res = run_bass_kernel_spmd(nc, in_maps, core_ids=list(range(N_CORES)))
    r0 = res.results[0]
    out = r0["out_sm"].reshape(1, O)
    h_new = np.concatenate(
        [res.results[r]["h_new_shard"][0] for r in range(N_CORES)]).reshape(1, 1, H)
    att_w = r0["att_w"].reshape(1, MAX_LEN)
    return out, h_new, att_w


def _exec_setup(in_maps):
    """Build a reusable jitted executable with device-resident inputs."""
    import jax
    from jax.sharding import Mesh, NamedSharding, PartitionSpec
    from jax.experimental.shard_map import shard_map
    from concourse import bass2jax, mybir as _mybir

    nc = _get_nc()
    bass2jax.install_neuronx_cc_hook()
    partition_name = nc.partition_id_tensor.name if nc.partition_id_tensor else None
    in_names, out_names, out_avals, zero_outs = [], [], [], []
    for alloc in nc.m.functions[0].allocations:
        if not isinstance(alloc, _mybir.MemoryLocationSet):
            continue
        name = alloc.memorylocations[0].name
        if alloc.kind == "ExternalInput":
            if name != partition_name:
                in_names.append(name)
        elif alloc.kind == "ExternalOutput":
            shape = tuple(alloc.tensor_shape)
            dtype = _mybir.dt.np(alloc.dtype)
            out_names.append(name)
            out_avals.append(jax.core.ShapedArray(shape, dtype))
            zero_outs.append(np.zeros(shape, dtype))
    n_params = len(in_names)
    all_in_names = list(in_names) + list(out_names)
    if partition_name is not None:
        all_in_names.append(partition_name)

    def _body(*args):
        operands = list(args)
        if partition_name is not None:
            operands.append(bass2jax.partition_id_tensor())
        outs = bass2jax._bass_exec_p.bind(
            *operands,
            out_avals=tuple(out_avals),
            in_names=tuple(all_in_names),
            out_names=tuple(out_names),
            lowering_input_output_aliases=(),
            sim_require_finite=True,
            sim_require_nnan=True,
            nc=nc,
        )
        return tuple(outs)

    devices = jax.devices()[:N_CORES]
    mesh = Mesh(np.asarray(devices), ("core",))
    n_all = n_params + len(zero_outs)
    sharded = jax.jit(
        shard_map(_body, mesh=mesh,
                  in_specs=(PartitionSpec("core"),) * n_all,
                  out_specs=(PartitionSpec("core"),) * len(out_names),
                  check_rep=False),
        keep_unused=True)
    sh = NamedSharding(mesh, PartitionSpec("core"))
    concat_in = [
        np.concatenate([np.asarray(in_maps[c][n]) for c in range(N_CORES)], axis=0)
        for n in in_names
    ]
    concat_zeros = [np.zeros((N_CORES * z.shape[0], *z.shape[1:]), z.dtype)
                    for z in zero_outs]
    dev_in = [jax.device_put(a, sh) for a in concat_in + concat_zeros]
    return sharded, dev_in, out_names


def bench(inputs, iters=12):
    """Estimate per-execution device time (ns) via async-chain slope."""
    import time as _time
    import jax
    in_maps = _prepare_in_maps(
        np.asarray(inputs["input_ids"]), np.asarray(inputs["hidden"]),
        np.asarray(inputs["e_output_seq"]), np.asarray(inputs["emb_W"]),
        np.asarray(inputs["att_W"]), np.asarray(inputs["att_b"]),
        np.asarray(inputs["comb_W"]), np.asarray(inputs["comb_b"]),
        np.asarray(inputs["gru_W_ih"]), np.asarray(inputs["gru_W_hh"]),
        np.asarray(inputs["gru_b_ih"]), np.asarray(inputs["gru_b_hh"]),
        np.asarray(inputs["out_W"]), np.asarray(inputs["out_b"]))
    sharded, dev_in, _ = _exec_setup(in_maps)

    def chain(k):
        r = None
        t0 = _time.perf_counter()
        for _ in range(k):
            r = sharded(*dev_in)
        jax.block_until_ready(r)
        return _time.perf_counter() - t0

    chain(3)  # warm up compile + caches
    k1, k2 = 8, 8 + max(4, iters)
    t1 = min(chain(k1) for _ in range(3))
    t2 = min(chain(k2) for _ in range(3))
    per_exec = (t2 - t1) / (k2 - k1)
    print(f"[bench] chain({k1})={t1*1e3:.2f}ms chain({k2})={t2*1e3:.2f}ms "
          f"-> per-exec {per_exec*1e6:.1f}us")
    return per_exec * 1e9


if __name__ == "__main__":
    rng = np.random.default_rng(0)
    s = 0.02
    inp = dict(
        input_ids=np.array([7], np.int64),
        hidden=rng.normal(size=(1, 1, H)).astype(np.float32),
        e_output=rng.normal(size=(1, H)).astype(np.float32),
        e_output_seq=rng.normal(size=(MAX_LEN, H)).astype(np.float32),
        emb_W=(rng.normal(size=(H, O)) * s).astype(np.float32),
        att_W=(rng.normal(size=(MAX_LEN, H + O)) * s).astype(np.float32),
        att_b=np.zeros(MAX_LEN, np.float32),
        comb_W=(rng.normal(size=(H, H + O)) * s).astype(np.float32),
        comb_b=np.zeros(H, np.float32),
        gru_W_ih=(rng.normal(size=(3 * H, H)) * s).astype(np.float32),
        gru_W_hh=(rng.normal(size=(3 * H, H)) * s).astype(np.float32),
        gru_b_ih=np.zeros(3 * H, np.float32),
        gru_b_hh=np.zeros(3 * H, np.float32),
        out_W=(rng.normal(size=(O, H)) * s).astype(np.float32),
        out_b=np.zeros(O, np.float32),
    )
    out, h_new, att_w = kernel(**inp)

    # numpy reference
    def np_ref(i):
        emb = i["emb_W"][int(i["input_ids"][0])][None, :]
        h0 = i["hidden"][0]
        al = np.concatenate([emb, h0], 1) @ i["att_W"].T + i["att_b"]
        aw = np.exp(al - al.max())
        aw = aw / aw.sum()
        aa = aw @ i["e_output_seq"]
        comb = np.concatenate([emb, aa], 1) @ i["comb_W"].T + i["comb_b"]
        x = np.maximum(comb, 0)
        gi = x @ i["gru_W_ih"].T + i["gru_b_ih"]
        gh = h0 @ i["gru_W_hh"].T + i["gru_b_hh"]
        ir, iz, inn = np.split(gi, 3, 1)
        hr, hz, hn = np.split(gh, 3, 1)
        r = 1 / (1 + np.exp(-(ir + hr)))
        z = 1 / (1 + np.exp(-(iz + hz)))
        n = np.tanh(inn + r * hn)
        hnew = (1 - z) * n + z * h0
        lo = hnew @ i["out_W"].T + i["out_b"]
        ls = lo - (np.log(np.exp(lo - lo.max()).sum()) + lo.max())
        return ls, hnew[None], aw

    eo, eh, ea = np_ref({k: np.asarray(v, np.float64) for k, v in inp.items()})
    for name, got, exp in (("out", out, eo), ("h_new", h_new, eh), ("att_w", att_w, ea)):
        err = np.abs(got - exp).max() / (np.abs(exp).max() + 1e-30)
        print(f"{name}: rel err {err:.3e}")
